# revision 37
# baseline (speedup 1.0000x reference)
"""Multi-head attention (B=2, S=2048, D=1024, H=16) on 8 NeuronCores.

Sharding: core = (batch b, head-group hg) with b in {0,1}, hg in {0..3}.
Each core computes 4 heads (256 of the 1024 hidden dims) for one batch
element and produces a partial output [S, D]; the host sums the 4
head-group partials per batch and adds the output bias.

Per-core dataflow:
  Q^T = Wq_c^T @ x^T  (bf16 matmuls, fp32 PSUM) -> quantized fp8e4 on the
        PSUM->SBUF bias-add copy, laid out [128p=(h,dh), 2=dp, S] so the
        scores matmuls can run in fp8 DoubleRow mode (d = 2*dh + dp).
  K^T likewise.
  V   = x @ Wv  (no bias)   [S, 256] bf16
  scores^T[k,q] = per (head, key-tile): ONE fp8e4 DoubleRow matmul
        lhsT = K^T[32, 2, 128], rhs = Q^T[32, 2, 512]  (Ki=32 x 2 planes
        = 64-dim contraction; half cost per output row vs bf16)
  attn^T = exp(scores^T / 8) bf16 (no max subtraction: |s/8| < ~2)
  ctx_aug^T = [ones | V_h]^T @ attn^T  (bf16) -> row 0 = softmax denom
  ctx^T = ctx_aug^T * partition_broadcast(1/denom)
  out_partial = sum_h ctx_h^T slices @ Wo_aug_h  (K=65; Wo row 0 carries
        bv_h @ Wo_h so the ctx 1.0-row adds the V-bias contribution)
Partial outputs are bf16; the host accumulates head groups + bo in fp32.

Host-side input layouts (pre-tiled so every load is one plain 2D DMA):
  xT  [1024, 2048]  x[b].T                                       bf16
  wq/wk [2, 128, 1024]  m-tile-split, k-tile-major columns; the
        columns of each m-tile mt are ordered (h, dh) -> Wq column
        64*h + 2*dh + mt, so the projection PSUM rows land directly in
        the (h, dh) partition layout with mt as the dp plane.         bf16
  wv  [128, 2048]   k-tile-major columns                         bf16
  wo  [65, 4096]    per-head [bv_h @ Wo_h; Wo_h] side by side    bf16
  bq/bk [128, 2]    bias m-tile columns (same column order)      f32
"""

from contextlib import ExitStack

import ml_dtypes
import numpy as np

import concourse.bass as bass
import concourse.mybir as mybir
import concourse.tile as tile
from concourse import bacc
from concourse.bass import ts
from concourse import bass_utils

S = 2048
D = 1024
H = 16
HD = 64
HPC = 4          # heads per core
C = HPC * HD     # 256 hidden dims per core
N_CORES = 8

BF16 = mybir.dt.bfloat16
F32 = mybir.dt.float32
F8 = mybir.dt.float8e4
NP_BF16 = ml_dtypes.bfloat16
NP_F8 = np.dtype(mybir.dt.np(mybir.dt.float8e4))
DR = mybir.MatmulPerfMode.DoubleRow

_CACHE = {}


def _build_nc():
    nc = bacc.Bacc(
        "TRN2", target_bir_lowering=False, debug=False, num_devices=N_CORES
    )

    xT = nc.dram_tensor("xT", [D, S], BF16, kind="ExternalInput").ap()
    wq = nc.dram_tensor("wq", [2, 128, 8 * 128], BF16, kind="ExternalInput").ap()
    wk = nc.dram_tensor("wk", [2, 128, 8 * 128], BF16, kind="ExternalInput").ap()
    wv = nc.dram_tensor("wv", [128, 8 * C], BF16, kind="ExternalInput").ap()
    wo = nc.dram_tensor("wo", [HD + 1, HPC * D], BF16, kind="ExternalInput").ap()
    bq = nc.dram_tensor("bq", [128, 2], F32, kind="ExternalInput").ap()
    bk = nc.dram_tensor("bk", [128, 2], F32, kind="ExternalInput").ap()
    out = nc.dram_tensor("out", [S, D], BF16, kind="ExternalOutput").ap()
    # chunk-3 output rows split by head pair: out3b carries the heads-2/3
    # contribution for rows 1536:2048 (host adds it), so the heads-0/1 half
    # of the final out-projection can run before the last norms complete.
    out3b = nc.dram_tensor("out3b", [512, D], BF16, kind="ExternalOutput").ap()

    with tile.TileContext(nc, pool_alloc_mode="queue") as tc, ExitStack() as ctx:
        ep = ctx.enter_context

        xt_pool = ep(tc.tile_pool(name="xt", bufs=8))
        w_pool = ep(tc.tile_pool(name="w", bufs=5))
        wo_pool = ep(tc.tile_pool(name="wo", bufs=1))
        small_pool = ep(tc.tile_pool(name="small", bufs=4))
        qk_pool = ep(tc.tile_pool(name="qk", bufs=2))
        vaug_pool = ep(tc.tile_pool(name="vaug", bufs=16))
        ctx_pool = ep(tc.tile_pool(name="ctxp", bufs=16))
        attn_pool = ep(tc.tile_pool(name="attn", bufs=38))
        recip_pool = ep(tc.tile_pool(name="recip", bufs=4))
        bcast_pool = ep(tc.tile_pool(name="bcast", bufs=4))
        outsb_pool = ep(tc.tile_pool(name="outsb", bufs=4))
        mm_ps = ep(tc.tile_pool(name="mmps", bufs=2, space="PSUM"))
        sc_ps = ep(tc.tile_pool(name="scps", bufs=2, space="PSUM"))
        cx_ps = ep(tc.tile_pool(name="cxps", bufs=2, space="PSUM"))

        # ---- loads (weights first; xT in k-tiles) ----
        # The first consumers (K00.k0, K01.k0) need only the k=0 weight
        # columns and the first/second q-chunks of xt0, so those land as
        # small head DMAs ahead of the bulk transfers: the first matmul
        # starts ~2us earlier than with whole-tile loads.
        # The warmup K/Q rounds read only columns 0:1024 of each xt k-tile
        # (q-chunks 0-1), so each xt loads as an a-half (critical) and a
        # b-half (deferred until after all a-halves): the warmup-critical
        # DMA stream shrinks from ~13.5us to ~8.6us.
        wk_sb = [None, None]
        wq_sb = [None, None]
        # the first transfers issue from BOTH hwdge queues (SP + ACT) so
        # their fixed per-DMA pipeline heads overlap
        for m in range(2):
            wk_sb[m] = w_pool.tile([128, 8 * 128], BF16, tag="w", name=f"wk_sb{m}")
            nc.sync.dma_start(wk_sb[m][:, 0:128], wk[m][:, 0:128])
        xt = [xt_pool.tile([128, S], BF16, tag="xt", name=f"xt_{k}") for k in range(8)]
        nc.scalar.dma_start(xt[0][:, 0:1024], xT[ts(0, 128), 0:1024])
        for m in range(2):
            nc.sync.dma_start(wk_sb[m][:, 128:1024], wk[m][:, 128:1024])
        nc.sync.dma_start(xt[1][:, 0:1024], xT[ts(1, 128), 0:1024])
        bk_sb = small_pool.tile([128, 2], F32, tag="bqk", name="bk_sb")
        nc.sync.dma_start(bk_sb[:], bk[:])
        bq_sb = small_pool.tile([128, 2], F32, tag="bqk", name="bq_sb")
        nc.sync.dma_start(bq_sb[:], bq[:])
        # wq heads cover k-steps 0-1; Q warmup rounds lag 3 k-steps so the
        # wq head/bulk transfers can trail the early xt a-halves without
        # stalling the PE
        for m in range(2):
            wq_sb[m] = w_pool.tile([128, 8 * 128], BF16, tag="w", name=f"wq_sb{m}")
            nc.scalar.dma_start(wq_sb[m][:, 0:256], wq[m][:, 0:256])
        for k in range(2, 5):
            nc.sync.dma_start(xt[k][:, 0:1024], xT[ts(k, 128), 0:1024])
        for m in range(2):
            nc.sync.dma_start(wq_sb[m][:, 256:1024], wq[m][:, 256:1024])
        for k in range(5, 8):
            nc.sync.dma_start(xt[k][:, 0:1024], xT[ts(k, 128), 0:1024])
        wv_sb = w_pool.tile([128, 8 * C], BF16, tag="w", name="wv_sb")
        nc.sync.dma_start(wv_sb[:], wv[:])
        for k in range(8):
            nc.sync.dma_start(xt[k][:, 1024:2048], xT[ts(k, 128), 1024:2048])
        wo_sb = wo_pool.tile([HD + 1, HPC * D], BF16, tag="wo", name="wo_sb")
        nc.sync.dma_start(wo_sb[:], wo[:])

        # fp8 K^T/Q^T tiles: [128 = (h, dh), 2 = dp, 2048 = s]
        kt = qk_pool.tile([128, 2 * S], F8, tag="qk", name="kt")
        qt = qk_pool.tile([128, 2 * S], F8, tag="qk", name="qt")
        kt3 = kt[:].rearrange("p (two s) -> p two s", two=2)
        qt3 = qt[:].rearrange("p (two s) -> p two s", two=2)

        # ---- projection emitters ----
        def emit_kq_round(dst3, w_t, b_t, m, n, label, pool=None, tag="mm"):
            ps = (pool or mm_ps).tile(
                [128, 512], F32, tag=tag, name=f"ps{label}_{m}_{n}"
            )
            for k in range(8):
                nc.tensor.matmul(
                    ps[:],
                    lhsT=w_t[m][:, ts(k, 128)],
                    rhs=xt[k][:, ts(n, 512)],
                    start=(k == 0),
                    stop=(k == 7),
                )
            nc.vector.tensor_scalar(
                dst3[:, m, ts(n, 512)],
                ps[:],
                b_t[:, m : m + 1],
                None,
                mybir.AluOpType.add,
            )

        vaug = []

        # ---- attention unit: one (q-chunk n, head h) ----
        ctx_tiles = {}

        def emit_scores(n, h, j, pool=None, tag="sc"):
            """fp8 DoubleRow scores for key tiles t=2j,2j+1 + exp -> at."""
            sc = (pool or sc_ps).tile(
                [128, 1024], F32, tag=tag, name=f"sc_{n}_{h}_{j}"
            )
            for tt in range(2):
                t = 2 * j + tt
                nc.tensor.matmul(
                    sc[:, ts(tt, 512)],
                    lhsT=kt3[32 * h : 32 * h + 32, :, ts(t, 128)],
                    rhs=qt3[32 * h : 32 * h + 32, :, ts(n, 512)],
                    start=True,
                    stop=True,
                    perf_mode=DR,
                    tile_position=(32 * h, 0),
                )
            at = attn_pool.tile([128, 1024], BF16, tag="at", name=f"at_{n}_{h}_{j}")
            nc.scalar.activation(
                at[:],
                sc[:],
                mybir.ActivationFunctionType.Exp,
                scale=0.125,
            )
            return at

        def emit_ctx_mm(n, h, j, at, cx):
            for tt in range(2):
                t = 2 * j + tt
                nc.tensor.matmul(
                    cx[:],
                    lhsT=vaug[t][:, 65 * h : 65 * h + 65],
                    rhs=at[:, ts(tt, 512)],
                    start=(t == 0),
                    stop=(t == 15),
                )

        ones65 = small_pool.tile([1, HD + 1], F32, tag="ones", name="ones65")
        nc.vector.memset(ones65[:], 1.0)
        F32R = mybir.dt.float32r

        def emit_norm(n, h, cx, pe_bcast=False):
            rc = recip_pool.tile([1, 512], F32, tag="rc", name=f"rc_{n}_{h}")
            nc.vector.reciprocal(rc[:], cx[0:1, :])
            if pe_bcast:
                # tail units: broadcast via a K=1 fp32r matmul on the (idle)
                # PE instead of GPSIMD — shorter critical chain into the
                # final out-projection.
                bc = mm_ps.tile([HD + 1, 512], F32, tag="mm", name=f"bc_{n}_{h}")
                nc.tensor.matmul(
                    bc[:],
                    lhsT=ones65[:].bitcast(F32R),
                    rhs=rc[:].bitcast(F32R),
                    start=True,
                    stop=True,
                )
            else:
                bc = bcast_pool.tile(
                    [HD + 1, 512], F32, tag="bc", name=f"bc_{n}_{h}"
                )
                nc.gpsimd.partition_broadcast(bc[:], rc[:], channels=HD + 1)
            ct = ctx_pool.tile([HD + 1, 512], BF16, tag="ctx", name=f"ctx_{n}_{h}")
            nc.vector.tensor_mul(ct[:], cx[:], bc[:])
            ctx_tiles[(h, n)] = ct

        ob_open = {}

        def emit_outproj_half(n, si, nn, act_copies=False, heads=range(HPC),
                              dst=None, key=None):
            """One outproj psum group (853ns PE): rows s=4n+si, D-half nn,
            summed over `heads`, written to dram `dst` (default: out).
            The output row-block DMAs in two half-width transfers so the
            tail drain starts as soon as the first half's copy lands."""
            s = 4 * n + si
            if dst is None:
                dst = out[ts(s, 128), :]
            key = (key, s)
            if nn == 0:
                ob_open[key] = outsb_pool.tile(
                    [128, D], BF16, tag="ob", name=f"ob_{key[0]}_{s}"
                )
            ob = ob_open[key]
            pool, tag = (mm_ps, "mm")
            if act_copies:
                pool, tag = (sc_ps, "sc") if nn == 0 else (cx_ps, "cx")
            ps = pool.tile([128, 512], F32, tag=tag, name=f"pso_{key[0]}_{s}_{nn}")
            heads = list(heads)
            for h in heads:
                nc.tensor.matmul(
                    ps[:],
                    lhsT=ctx_tiles[(h, n)][:, ts(si, 128)],
                    rhs=wo_sb[
                        :, 1024 * h + 512 * nn : 1024 * h + 512 * nn + 512
                    ],
                    start=(h == heads[0]),
                    stop=(h == heads[-1]),
                )
            if act_copies and nn == 1:
                nc.scalar.copy(ob[:, ts(nn, 512)], ps[:])
            else:
                nc.vector.tensor_copy(ob[:, ts(nn, 512)], ps[:])
            # one full-width DMA per row block: the DMA queue is per-transfer
            # overhead-dominated, so fewer/bigger beats earlier/smaller —
            # except in the drain tail, where half-DMAs let the first half
            # fly while the second half's matmuls still run
            if act_copies:
                nc.sync.dma_start(dst[:, ts(nn, 512)], ob[:, ts(nn, 512)])
                if nn == 1:
                    del ob_open[key]
            elif nn == 1:
                nc.sync.dma_start(dst[:], ob[:])
                del ob_open[key]

        # ---- emission order ----
        # ACT table-load warm: dummy exp as soon as wk0 lands, so the ~1.3us
        # table load overlaps the xT DMA stream instead of the first scores.
        warm = small_pool.tile([1, 8], BF16, tag="warm", name="actwarm")
        nc.scalar.activation(
            warm[:],
            wk_sb[0][0:1, 0:8],
            mybir.ActivationFunctionType.Exp,
            scale=0.125,
        )

        # Warmup: six K/Q rounds accumulate k-MAJOR so the PE tracks the xT
        # DMA stream (6 matmuls ready per xt tile arrival). Q rounds lag one
        # k-step because wq lands after xt0.
        wu = [
            ("K00", wk_sb[0], kt3, bk_sb, 0, 0, mm_ps, "mm"),
            ("K10", wk_sb[1], kt3, bk_sb, 1, 0, cx_ps, "cx"),
            ("K01", wk_sb[0], kt3, bk_sb, 0, 1, mm_ps, "mm"),
            ("K11", wk_sb[1], kt3, bk_sb, 1, 1, cx_ps, "cx"),
            ("Q00", wq_sb[0], qt3, bq_sb, 0, 0, sc_ps, "sc"),
            ("Q10", wq_sb[1], qt3, bq_sb, 1, 0, sc_ps, "sc"),
        ]
        wu_ps = {
            nm: pool.tile([128, 512], F32, tag=tag, name=f"wu{nm}")
            for (nm, _, _, _, _, _, pool, tag) in wu
        }
        for k in range(11):
            for nm, w_t, _, _, m, n, _, _ in wu:
                kk = k - 3 if nm[0] == "Q" else k
                if not (0 <= kk < 8):
                    continue
                nc.tensor.matmul(
                    wu_ps[nm][:],
                    lhsT=w_t[:, ts(kk, 128)],
                    rhs=xt[kk][:, ts(n, 512)],
                    start=(kk == 0),
                    stop=(kk == 7),
                )
        # bias-add copies split across DVE and ACT so the first scores
        # (which need K00/K10/Q00/Q10) unblock after ~2 copies per engine.
        wu_cp_order = {"K00": 0, "K10": 1, "Q00": 2, "Q10": 3, "K01": 4, "K11": 5}
        for nm, _, dst3, b_t, m, n, _, _ in sorted(
            wu, key=lambda e: wu_cp_order[e[0]]
        ):
            dst = dst3[:, m, ts(n, 512)]
            if nm in ("K10", "Q10", "K01"):
                nc.scalar.activation(
                    dst,
                    wu_ps[nm][:],
                    mybir.ActivationFunctionType.Identity,
                    bias=b_t[:, m : m + 1],
                )
            else:
                nc.vector.tensor_scalar(
                    dst,
                    wu_ps[nm][:],
                    b_t[:, m : m + 1],
                    None,
                    mybir.AluOpType.add,
                )

        # Chunk-0 scores stream with the remaining 10 K/Q rounds and the 16
        # V rounds as PE backfill between ACT-gated score tiles.
        ats0 = {h: [] for h in range(HPC)}
        kq_backfill = [
            ("q", 0, 1), ("q", 1, 1),
            ("k", 0, 2), ("k", 1, 2),
            ("q", 0, 2), ("q", 1, 2),
            ("k", 0, 3), ("k", 1, 3),
            ("q", 0, 3), ("q", 1, 3),
        ]
        v_emitted = 0

        def emit_v_round(s):
            ps = mm_ps.tile([128, C], F32, tag="mm", name=f"psv_{s}")
            for k in range(8):
                nc.tensor.matmul(
                    ps[:],
                    lhsT=xt[k][:, ts(s, 128)],
                    rhs=wv_sb[:, ts(k, C)],
                    start=(k == 0),
                    stop=(k == 7),
                )
            vt = vaug_pool.tile(
                [128, HPC * (HD + 1)], BF16, tag="vaug", name=f"vaug_{s}"
            )
            vt3 = vt[:].rearrange("p (h x) -> p h x", x=HD + 1)
            nc.vector.memset(vt3[:, :, 0:1], 1.0)
            nc.vector.tensor_copy(
                vt3[:, :, 1 : HD + 1],
                ps[:].rearrange("p (h d) -> p h d", d=HD),
            )
            vaug.append(vt)

        bf_i = 0

        def pop_backfill():
            nonlocal bf_i, v_emitted
            if bf_i < len(kq_backfill):
                kind, m, nn = kq_backfill[bf_i]
                pool, tag = (mm_ps, "mm") if bf_i % 2 == 0 else (cx_ps, "cx")
                if kind == "k":
                    emit_kq_round(kt3, wk_sb, bk_sb, m, nn, "k", pool=pool, tag=tag)
                else:
                    emit_kq_round(qt3, wq_sb, bq_sb, m, nn, "q", pool=pool, tag=tag)
                bf_i += 1
            elif v_emitted < 16:
                emit_v_round(v_emitted)
                v_emitted += 1

        # K-round backfill for kt chunk nn pops before the j = 2*nn scores
        # need it.  The last 4 V rounds move into the main stream's piece
        # queue so the chunk-0 phase tail doesn't starve ACT.
        for j in range(8):
            for h in range(HPC):
                ats0[h].append(emit_scores(0, h, j))
                if h % 2 == 1:
                    pop_backfill()
        while bf_i < len(kq_backfill) or v_emitted < 12:
            pop_backfill()

        # Chunks 1-3: software-pipelined scores/ctx (ctx lags 2 score tiles
        # so exp has drained), with chunk-0 ctx and deferred outproj pieces
        # as additional PE backfill spread through the stream.
        from collections import deque

        cx_cur = {}

        def pop_ctx(pend):
            n, h, j, at = pend.popleft()
            if j == 0:
                cx_cur[(n, h)] = cx_ps.tile(
                    [HD + 1, 512], F32, tag="cx", name=f"cx_{n}_{h}"
                )
            emit_ctx_mm(n, h, j, at, cx_cur[(n, h)])
            if j == 7:
                emit_norm(n, h, cx_cur.pop((n, h)))

        # chunk-0 ctx units as backfill pieces for chunk 1 (quarter-unit per
        # piece, 4 matmuls ~850ns); they are all exp-complete by now.
        ctx0_pieces = []
        for h in range(HPC):
            for qt_ in range(4):
                ctx0_pieces.append((h, qt_))

        def emit_ctx0_piece():
            # ctx0 backfills chunk 1, whose pair-interleaved units occupy
            # both cx slots — use the (idle in chunk 1) mm pool instead.
            h, qt_ = ctx0_pieces.pop(0)
            if qt_ == 0:
                cx_cur[(0, h)] = mm_ps.tile(
                    [HD + 1, 512], F32, tag="mm", name=f"cx_0_{h}"
                )
            for j in range(2 * qt_, 2 * qt_ + 2):
                emit_ctx_mm(0, h, j, ats0[h][j], cx_cur[(0, h)])
            if qt_ == 3:
                emit_norm(0, h, cx_cur.pop((0, h)))

        # Chunks 1-3 run as ONE continuous 96-step stream (no chunk seams):
        # ctx pops lag 5 score tiles, and the backfill queue [ctx0 pieces,
        # outproj(0..2) halves] fires evenly across the whole stream.  Each
        # piece kind becomes data-ready just before its queue position.
        # The heads-0/1 half of chunk-3's outproj fires in the last stream
        # steps (norms(3,0/1) complete ~15 steps before the end) targeting
        # out3b, leaving only the heads-2/3 half for the tail.
        pend = deque()
        pieces = [
            (lambda s=s: emit_v_round(s)) for s in range(12, 16)
        ] + [emit_ctx0_piece] * len(ctx0_pieces)
        for pn in range(3):
            pieces += [
                (lambda si=si, nn=nn, pn=pn: emit_outproj_half(pn, si, nn))
                for si in range(4)
                for nn in range(2)
            ]
        late_pieces = [
            (lambda si=si, nn=nn: emit_outproj_half(
                3, si, nn, heads=(0, 1), dst=out3b[ts(si, 128), :], key="b"))
            for si in range(4)
            for nn in range(2)
        ]
        npieces = len(pieces)
        nsteps = 96
        fired = 0
        step = 0
        # units run in interleaved PAIRS so a unit's norm chain (recip ->
        # broadcast -> mul, ~2.5us) overlaps the partner unit's stream
        # instead of stalling the next cx psum allocation.
        for n in range(1, 4):
            for hp in range(2):
                for j in range(8):
                    for h in (2 * hp, 2 * hp + 1):
                        at = emit_scores(n, h, j)
                        pend.append((n, h, j, at))
                        if len(pend) > 5:
                            pop_ctx(pend)
                        # drain the ctx pipeline faster near the very end so
                        # the final norms complete earlier
                        if step >= nsteps - 8 and pend:
                            pop_ctx(pend)
                        step += 1
                        while pieces and fired < step * npieces // (nsteps - 8):
                            pieces.pop(0)()
                            fired += 1
                        if step >= 89 and late_pieces:
                            late_pieces.pop(0)()
        while pend:
            pop_ctx(pend)
        while pieces:
            pieces.pop(0)()
        while late_pieces:
            late_pieces.pop(0)()
        for si in range(4):
            for nn in range(2):
                emit_outproj_half(3, si, nn, act_copies=True, heads=(2, 3))

    nc.compile()
    return nc


def _get_nc():
    if "nc" not in _CACHE:
        _CACHE["nc"] = _build_nc()
    return _CACHE["nc"]


def _make_in_maps(inputs):
    x = np.asarray(inputs["x"], np.float32)
    Wq = np.asarray(inputs["Wq"], np.float32)
    Wk = np.asarray(inputs["Wk"], np.float32)
    Wv = np.asarray(inputs["Wv"], np.float32)
    Wo = np.asarray(inputs["Wo"], np.float32)
    bq = np.asarray(inputs["bq"], np.float32)
    bk = np.asarray(inputs["bk"], np.float32)
    bv = np.asarray(inputs["bv"], np.float32)

    # column permutation for the (h, dh, dp) projection layout:
    # m-tile mt, partition r=(h*32+dh) -> local column 64*h + 2*dh + mt
    r = np.arange(128)
    perm = np.concatenate(
        [64 * (r // 32) + 2 * (r % 32) + mt for mt in range(2)]
    )  # [256] local column index, m-tile-major

    def tile_w(w_slice):
        # [1024, 256] -> permute columns -> [2, 128, 8*128]
        wp = w_slice[:, perm]
        return np.ascontiguousarray(
            wp.reshape(8, 128, 2, 128).transpose(2, 1, 0, 3).reshape(2, 128, 8 * 128)
        ).astype(NP_BF16)

    def tile_b(b_slice):
        return np.ascontiguousarray(b_slice[perm].reshape(2, 128).T)

    def tile_wv(w_slice):
        # [1024, 256] -> [128, 8*256] with k-tile-major free dim
        return np.ascontiguousarray(
            w_slice.reshape(8, 128, C).transpose(1, 0, 2).reshape(128, 8 * C)
        ).astype(NP_BF16)

    in_maps = []
    for core in range(N_CORES):
        b, hg = core // 4, core % 4
        cs = slice(C * hg, C * (hg + 1))
        xT = np.ascontiguousarray(x[b].T).astype(NP_BF16)
        wo_c = np.zeros((HD + 1, HPC * D), np.float32)
        for h in range(HPC):
            r0 = C * hg + HD * h
            wo_c[1 : HD + 1, D * h : D * (h + 1)] = Wo[r0 : r0 + HD]
            wo_c[0, D * h : D * (h + 1)] = bv[r0 : r0 + HD] @ Wo[r0 : r0 + HD]
        in_maps.append(
            {
                "xT": xT,
                "wq": tile_w(Wq[:, cs]),
                "wk": tile_w(Wk[:, cs]),
                "wv": tile_wv(Wv[:, cs]),
                "wo": wo_c.astype(NP_BF16),
                "bq": tile_b(bq[cs]),
                "bk": tile_b(bk[cs]),
            }
        )
    return in_maps


def run(inputs, trace=False):
    """Run the SPMD kernel; returns (full_output, BassKernelResults)."""
    nc = _get_nc()
    in_maps = _make_in_maps(inputs)
    res = bass_utils.run_bass_kernel_spmd(
        nc, in_maps, core_ids=list(range(N_CORES)), trace=trace
    )
    bo = np.asarray(inputs["bo"], np.float32)
    full = np.empty((2, S, D), np.float32)
    for b in range(2):
        acc = res.results[4 * b]["out"].astype(np.float32).copy()
        acc[1536:2048] += res.results[4 * b]["out3b"]
        for hg in range(1, 4):
            acc += res.results[4 * b + hg]["out"]
            acc[1536:2048] += res.results[4 * b + hg]["out3b"]
        full[b] = acc + bo
    return full, res


def kernel(**inputs):
    full, _ = run(inputs, trace=False)
    return full


# revision 38
# speedup vs baseline: 1.0128x; 1.0128x over previous
"""Multi-head attention (B=2, S=2048, D=1024, H=16) on 8 NeuronCores.

Sharding: core = (batch b, head-group hg) with b in {0,1}, hg in {0..3}.
Each core computes 4 heads (256 of the 1024 hidden dims) for one batch
element and produces a partial output [S, D]; the host sums the 4
head-group partials per batch and adds the output bias.

Per-core dataflow:
  Q^T = Wq_c^T @ x^T  (bf16 matmuls, fp32 PSUM) -> quantized fp8e4 on the
        PSUM->SBUF bias-add copy, laid out [128p=(h,dh), 2=dp, S] so the
        scores matmuls can run in fp8 DoubleRow mode (d = 2*dh + dp).
  K^T likewise.
  V   = x @ Wv  (no bias)   [S, 256] bf16
  scores^T[k,q] = per (head, key-tile): ONE fp8e4 DoubleRow matmul
        lhsT = K^T[32, 2, 128], rhs = Q^T[32, 2, 512]  (Ki=32 x 2 planes
        = 64-dim contraction; half cost per output row vs bf16)
  attn^T = exp(scores^T / 8) bf16 (no max subtraction: |s/8| < ~2)
  ctx_aug^T = [ones | V_h]^T @ attn^T  (bf16) -> row 0 = softmax denom
  ctx^T = ctx_aug^T * partition_broadcast(1/denom)
  out_partial = sum_h ctx_h^T slices @ Wo_aug_h  (K=65; Wo row 0 carries
        bv_h @ Wo_h so the ctx 1.0-row adds the V-bias contribution)
Partial outputs are bf16; the host accumulates head groups + bo in fp32.

Host-side input layouts (pre-tiled so every load is one plain 2D DMA):
  xT  [1024, 2048]  x[b].T                                       bf16
  wq/wk [2, 128, 1024]  m-tile-split, k-tile-major columns; the
        columns of each m-tile mt are ordered (h, dh) -> Wq column
        64*h + 2*dh + mt, so the projection PSUM rows land directly in
        the (h, dh) partition layout with mt as the dp plane.         bf16
  wv  [128, 2048]   k-tile-major columns                         bf16
  wo  [65, 4096]    per-head [bv_h @ Wo_h; Wo_h] side by side    bf16
  bq/bk [128, 2]    bias m-tile columns (same column order)      f32
"""

from contextlib import ExitStack

import ml_dtypes
import numpy as np

import concourse.bass as bass
import concourse.mybir as mybir
import concourse.tile as tile
from concourse import bacc
from concourse.bass import ts
from concourse import bass_utils

S = 2048
D = 1024
H = 16
HD = 64
HPC = 4          # heads per core
C = HPC * HD     # 256 hidden dims per core
N_CORES = 8

BF16 = mybir.dt.bfloat16
F32 = mybir.dt.float32
F8 = mybir.dt.float8e4
NP_BF16 = ml_dtypes.bfloat16
NP_F8 = np.dtype(mybir.dt.np(mybir.dt.float8e4))
DR = mybir.MatmulPerfMode.DoubleRow

_CACHE = {}


def _build_nc():
    nc = bacc.Bacc(
        "TRN2", target_bir_lowering=False, debug=False, num_devices=N_CORES
    )

    xT = nc.dram_tensor("xT", [D, S], BF16, kind="ExternalInput").ap()
    wq = nc.dram_tensor("wq", [2, 128, 8 * 128], BF16, kind="ExternalInput").ap()
    wk = nc.dram_tensor("wk", [2, 128, 8 * 128], BF16, kind="ExternalInput").ap()
    wv = nc.dram_tensor("wv", [128, 8 * C], BF16, kind="ExternalInput").ap()
    wo = nc.dram_tensor("wo", [HD + 1, HPC * D], BF16, kind="ExternalInput").ap()
    bq = nc.dram_tensor("bq", [128, 2], F32, kind="ExternalInput").ap()
    bk = nc.dram_tensor("bk", [128, 2], F32, kind="ExternalInput").ap()
    out = nc.dram_tensor("out", [S, D], BF16, kind="ExternalOutput").ap()
    # chunk-3 output rows split by head pair: out3b carries the heads-2/3
    # contribution for rows 1536:2048 (host adds it), so the heads-0/1 half
    # of the final out-projection can run before the last norms complete.
    out3b = nc.dram_tensor("out3b", [512, D], BF16, kind="ExternalOutput").ap()

    with tile.TileContext(nc, pool_alloc_mode="queue") as tc, ExitStack() as ctx:
        ep = ctx.enter_context

        xt_pool = ep(tc.tile_pool(name="xt", bufs=8))
        w_pool = ep(tc.tile_pool(name="w", bufs=5))
        wo_pool = ep(tc.tile_pool(name="wo", bufs=1))
        small_pool = ep(tc.tile_pool(name="small", bufs=4))
        qk_pool = ep(tc.tile_pool(name="qk", bufs=2))
        vaug_pool = ep(tc.tile_pool(name="vaug", bufs=16))
        ctx_pool = ep(tc.tile_pool(name="ctxp", bufs=16))
        attn_pool = ep(tc.tile_pool(name="attn", bufs=38))
        recip_pool = ep(tc.tile_pool(name="recip", bufs=4))
        bcast_pool = ep(tc.tile_pool(name="bcast", bufs=4))
        outsb_pool = ep(tc.tile_pool(name="outsb", bufs=4))
        mm_ps = ep(tc.tile_pool(name="mmps", bufs=2, space="PSUM"))
        sc_ps = ep(tc.tile_pool(name="scps", bufs=2, space="PSUM"))
        cx_ps = ep(tc.tile_pool(name="cxps", bufs=2, space="PSUM"))

        # ---- loads (weights first; xT in k-tiles) ----
        # The first consumers (K00.k0, K01.k0) need only the k=0 weight
        # columns and the first/second q-chunks of xt0, so those land as
        # small head DMAs ahead of the bulk transfers: the first matmul
        # starts ~2us earlier than with whole-tile loads.
        # The warmup K/Q rounds read only columns 0:1024 of each xt k-tile
        # (q-chunks 0-1), so each xt loads as an a-half (critical) and a
        # b-half (deferred until after all a-halves): the warmup-critical
        # DMA stream shrinks from ~13.5us to ~8.6us.
        wk_sb = [None, None]
        wq_sb = [None, None]
        # the first transfers issue from BOTH hwdge queues (SP + ACT) so
        # their fixed per-DMA pipeline heads overlap
        for m in range(2):
            wk_sb[m] = w_pool.tile([128, 8 * 128], BF16, tag="w", name=f"wk_sb{m}")
            nc.sync.dma_start(wk_sb[m][:, 0:128], wk[m][:, 0:128])
        xt = [xt_pool.tile([128, S], BF16, tag="xt", name=f"xt_{k}") for k in range(8)]
        nc.scalar.dma_start(xt[0][:, 0:1024], xT[ts(0, 128), 0:1024])
        for m in range(2):
            nc.sync.dma_start(wk_sb[m][:, 128:1024], wk[m][:, 128:1024])
        nc.sync.dma_start(xt[1][:, 0:1024], xT[ts(1, 128), 0:1024])
        bk_sb = small_pool.tile([128, 2], F32, tag="bqk", name="bk_sb")
        nc.sync.dma_start(bk_sb[:], bk[:])
        bq_sb = small_pool.tile([128, 2], F32, tag="bqk", name="bq_sb")
        nc.sync.dma_start(bq_sb[:], bq[:])
        # wq heads cover k-steps 0-1; Q warmup rounds lag 3 k-steps so the
        # wq head/bulk transfers can trail the early xt a-halves without
        # stalling the PE
        for m in range(2):
            wq_sb[m] = w_pool.tile([128, 8 * 128], BF16, tag="w", name=f"wq_sb{m}")
            nc.scalar.dma_start(wq_sb[m][:, 0:256], wq[m][:, 0:256])
        for k in range(2, 5):
            nc.sync.dma_start(xt[k][:, 0:1024], xT[ts(k, 128), 0:1024])
        for m in range(2):
            nc.sync.dma_start(wq_sb[m][:, 256:1024], wq[m][:, 256:1024])
        for k in range(5, 8):
            nc.sync.dma_start(xt[k][:, 0:1024], xT[ts(k, 128), 0:1024])
        wv_sb = w_pool.tile([128, 8 * C], BF16, tag="w", name="wv_sb")
        nc.sync.dma_start(wv_sb[:], wv[:])
        for k in range(8):
            nc.sync.dma_start(xt[k][:, 1024:2048], xT[ts(k, 128), 1024:2048])
        wo_sb = wo_pool.tile([HD + 1, HPC * D], BF16, tag="wo", name="wo_sb")
        nc.sync.dma_start(wo_sb[:], wo[:])

        # fp8 K^T/Q^T tiles: [128 = (h, dh), 2 = dp, 2048 = s]
        kt = qk_pool.tile([128, 2 * S], F8, tag="qk", name="kt")
        qt = qk_pool.tile([128, 2 * S], F8, tag="qk", name="qt")
        kt3 = kt[:].rearrange("p (two s) -> p two s", two=2)
        qt3 = qt[:].rearrange("p (two s) -> p two s", two=2)

        # ---- projection emitters ----
        def emit_kq_round(dst3, w_t, b_t, m, n, label, pool=None, tag="mm"):
            ps = (pool or mm_ps).tile(
                [128, 512], F32, tag=tag, name=f"ps{label}_{m}_{n}"
            )
            for k in range(8):
                nc.tensor.matmul(
                    ps[:],
                    lhsT=w_t[m][:, ts(k, 128)],
                    rhs=xt[k][:, ts(n, 512)],
                    start=(k == 0),
                    stop=(k == 7),
                )
            nc.vector.tensor_scalar(
                dst3[:, m, ts(n, 512)],
                ps[:],
                b_t[:, m : m + 1],
                None,
                mybir.AluOpType.add,
            )

        vaug = []

        # ---- attention unit: one (q-chunk n, head h) ----
        ctx_tiles = {}

        def emit_scores(n, h, j, pool=None, tag="sc"):
            """fp8 DoubleRow scores for key tiles t=2j,2j+1 + exp -> at."""
            sc = (pool or sc_ps).tile(
                [128, 1024], F32, tag=tag, name=f"sc_{n}_{h}_{j}"
            )
            for tt in range(2):
                t = 2 * j + tt
                nc.tensor.matmul(
                    sc[:, ts(tt, 512)],
                    lhsT=kt3[32 * h : 32 * h + 32, :, ts(t, 128)],
                    rhs=qt3[32 * h : 32 * h + 32, :, ts(n, 512)],
                    start=True,
                    stop=True,
                    perf_mode=DR,
                    tile_position=(32 * h, 0),
                )
            at = attn_pool.tile([128, 1024], BF16, tag="at", name=f"at_{n}_{h}_{j}")
            nc.scalar.activation(
                at[:],
                sc[:],
                mybir.ActivationFunctionType.Exp,
                scale=0.125,
            )
            return at

        def emit_ctx_mm(n, h, j, at, cx):
            for tt in range(2):
                t = 2 * j + tt
                nc.tensor.matmul(
                    cx[:],
                    lhsT=vaug[t][:, 65 * h : 65 * h + 65],
                    rhs=at[:, ts(tt, 512)],
                    start=(t == 0),
                    stop=(t == 15),
                )

        ones65 = small_pool.tile([1, HD + 1], F32, tag="ones", name="ones65")
        nc.vector.memset(ones65[:], 1.0)
        F32R = mybir.dt.float32r

        def emit_norm(n, h, cx, pe_bcast=False):
            rc = recip_pool.tile([1, 512], F32, tag="rc", name=f"rc_{n}_{h}")
            nc.vector.reciprocal(rc[:], cx[0:1, :])
            if pe_bcast:
                # tail units: broadcast via a K=1 fp32r matmul on the (idle)
                # PE instead of GPSIMD — shorter critical chain into the
                # final out-projection.
                bc = mm_ps.tile([HD + 1, 512], F32, tag="mm", name=f"bc_{n}_{h}")
                nc.tensor.matmul(
                    bc[:],
                    lhsT=ones65[:].bitcast(F32R),
                    rhs=rc[:].bitcast(F32R),
                    start=True,
                    stop=True,
                )
            else:
                bc = bcast_pool.tile(
                    [HD + 1, 512], F32, tag="bc", name=f"bc_{n}_{h}"
                )
                nc.gpsimd.partition_broadcast(bc[:], rc[:], channels=HD + 1)
            ct = ctx_pool.tile([HD + 1, 512], BF16, tag="ctx", name=f"ctx_{n}_{h}")
            nc.vector.tensor_mul(ct[:], cx[:], bc[:])
            ctx_tiles[(h, n)] = ct

        ob_open = {}

        def emit_outproj_half(n, si, nn, act_copies=False, heads=range(HPC),
                              dst=None, key=None):
            """One outproj psum group (853ns PE): rows s=4n+si, D-half nn,
            summed over `heads`, written to dram `dst` (default: out).
            The output row-block DMAs in two half-width transfers so the
            tail drain starts as soon as the first half's copy lands."""
            s = 4 * n + si
            if dst is None:
                dst = out[ts(s, 128), :]
            key = (key, s)
            if nn == 0:
                ob_open[key] = outsb_pool.tile(
                    [128, D], BF16, tag="ob", name=f"ob_{key[0]}_{s}"
                )
            ob = ob_open[key]
            pool, tag = (mm_ps, "mm")
            if act_copies:
                pool, tag = (sc_ps, "sc") if nn == 0 else (cx_ps, "cx")
            ps = pool.tile([128, 512], F32, tag=tag, name=f"pso_{key[0]}_{s}_{nn}")
            heads = list(heads)
            for h in heads:
                nc.tensor.matmul(
                    ps[:],
                    lhsT=ctx_tiles[(h, n)][:, ts(si, 128)],
                    rhs=wo_sb[
                        :, 1024 * h + 512 * nn : 1024 * h + 512 * nn + 512
                    ],
                    start=(h == heads[0]),
                    stop=(h == heads[-1]),
                )
            if act_copies and nn == 1:
                nc.scalar.copy(ob[:, ts(nn, 512)], ps[:])
            else:
                nc.vector.tensor_copy(ob[:, ts(nn, 512)], ps[:])
            # one full-width DMA per row block: the DMA queue is per-transfer
            # overhead-dominated, so fewer/bigger beats earlier/smaller
            if nn == 1:
                nc.sync.dma_start(dst[:], ob[:])
                del ob_open[key]

        # ---- emission order ----
        # ACT table-load warm: dummy exp as soon as wk0 lands, so the ~1.3us
        # table load overlaps the xT DMA stream instead of the first scores.
        warm = small_pool.tile([1, 8], BF16, tag="warm", name="actwarm")
        nc.scalar.activation(
            warm[:],
            wk_sb[0][0:1, 0:8],
            mybir.ActivationFunctionType.Exp,
            scale=0.125,
        )

        # Warmup: six K/Q rounds accumulate k-MAJOR so the PE tracks the xT
        # DMA stream (6 matmuls ready per xt tile arrival). Q rounds lag one
        # k-step because wq lands after xt0.
        wu = [
            ("K00", wk_sb[0], kt3, bk_sb, 0, 0, mm_ps, "mm"),
            ("K10", wk_sb[1], kt3, bk_sb, 1, 0, cx_ps, "cx"),
            ("K01", wk_sb[0], kt3, bk_sb, 0, 1, mm_ps, "mm"),
            ("K11", wk_sb[1], kt3, bk_sb, 1, 1, cx_ps, "cx"),
            ("Q00", wq_sb[0], qt3, bq_sb, 0, 0, sc_ps, "sc"),
            ("Q10", wq_sb[1], qt3, bq_sb, 1, 0, sc_ps, "sc"),
        ]
        wu_ps = {
            nm: pool.tile([128, 512], F32, tag=tag, name=f"wu{nm}")
            for (nm, _, _, _, _, _, pool, tag) in wu
        }
        for k in range(11):
            for nm, w_t, _, _, m, n, _, _ in wu:
                kk = k - 3 if nm[0] == "Q" else k
                if not (0 <= kk < 8):
                    continue
                nc.tensor.matmul(
                    wu_ps[nm][:],
                    lhsT=w_t[:, ts(kk, 128)],
                    rhs=xt[kk][:, ts(n, 512)],
                    start=(kk == 0),
                    stop=(kk == 7),
                )
        # bias-add copies split across DVE and ACT so the first scores
        # (which need K00/K10/Q00/Q10) unblock after ~2 copies per engine.
        wu_cp_order = {"K00": 0, "K10": 1, "Q00": 2, "Q10": 3, "K01": 4, "K11": 5}
        for nm, _, dst3, b_t, m, n, _, _ in sorted(
            wu, key=lambda e: wu_cp_order[e[0]]
        ):
            dst = dst3[:, m, ts(n, 512)]
            if nm in ("K10", "Q10", "K01"):
                nc.scalar.activation(
                    dst,
                    wu_ps[nm][:],
                    mybir.ActivationFunctionType.Identity,
                    bias=b_t[:, m : m + 1],
                )
            else:
                nc.vector.tensor_scalar(
                    dst,
                    wu_ps[nm][:],
                    b_t[:, m : m + 1],
                    None,
                    mybir.AluOpType.add,
                )

        # Chunk-0 scores stream with the remaining 10 K/Q rounds and the 16
        # V rounds as PE backfill between ACT-gated score tiles.
        ats0 = {h: [] for h in range(HPC)}
        kq_backfill = [
            ("q", 0, 1), ("q", 1, 1),
            ("k", 0, 2), ("k", 1, 2),
            ("q", 0, 2), ("q", 1, 2),
            ("k", 0, 3), ("k", 1, 3),
            ("q", 0, 3), ("q", 1, 3),
        ]
        v_emitted = 0

        def emit_v_round(s):
            ps = mm_ps.tile([128, C], F32, tag="mm", name=f"psv_{s}")
            for k in range(8):
                nc.tensor.matmul(
                    ps[:],
                    lhsT=xt[k][:, ts(s, 128)],
                    rhs=wv_sb[:, ts(k, C)],
                    start=(k == 0),
                    stop=(k == 7),
                )
            vt = vaug_pool.tile(
                [128, HPC * (HD + 1)], BF16, tag="vaug", name=f"vaug_{s}"
            )
            vt3 = vt[:].rearrange("p (h x) -> p h x", x=HD + 1)
            nc.vector.memset(vt3[:, :, 0:1], 1.0)
            nc.vector.tensor_copy(
                vt3[:, :, 1 : HD + 1],
                ps[:].rearrange("p (h d) -> p h d", d=HD),
            )
            vaug.append(vt)

        bf_i = 0

        def pop_backfill():
            nonlocal bf_i, v_emitted
            if bf_i < len(kq_backfill):
                kind, m, nn = kq_backfill[bf_i]
                pool, tag = (mm_ps, "mm") if bf_i % 2 == 0 else (cx_ps, "cx")
                if kind == "k":
                    emit_kq_round(kt3, wk_sb, bk_sb, m, nn, "k", pool=pool, tag=tag)
                else:
                    emit_kq_round(qt3, wq_sb, bq_sb, m, nn, "q", pool=pool, tag=tag)
                bf_i += 1
            elif v_emitted < 16:
                emit_v_round(v_emitted)
                v_emitted += 1

        # K-round backfill for kt chunk nn pops before the j = 2*nn scores
        # need it.  The last 4 V rounds move into the main stream's piece
        # queue so the chunk-0 phase tail doesn't starve ACT.
        for j in range(8):
            for h in range(HPC):
                ats0[h].append(emit_scores(0, h, j))
                if h % 2 == 1:
                    pop_backfill()
        while bf_i < len(kq_backfill) or v_emitted < 12:
            pop_backfill()

        # Chunks 1-3: software-pipelined scores/ctx (ctx lags 2 score tiles
        # so exp has drained), with chunk-0 ctx and deferred outproj pieces
        # as additional PE backfill spread through the stream.
        from collections import deque

        cx_cur = {}

        def pop_ctx(pend):
            n, h, j, at = pend.popleft()
            if j == 0:
                cx_cur[(n, h)] = cx_ps.tile(
                    [HD + 1, 512], F32, tag="cx", name=f"cx_{n}_{h}"
                )
            emit_ctx_mm(n, h, j, at, cx_cur[(n, h)])
            if j == 7:
                emit_norm(n, h, cx_cur.pop((n, h)))

        # chunk-0 ctx units as backfill pieces for chunk 1 (quarter-unit per
        # piece, 4 matmuls ~850ns); they are all exp-complete by now.
        ctx0_pieces = []
        for h in range(HPC):
            for qt_ in range(4):
                ctx0_pieces.append((h, qt_))

        def emit_ctx0_piece():
            # ctx0 backfills chunk 1, whose pair-interleaved units occupy
            # both cx slots — use the (idle in chunk 1) mm pool instead.
            h, qt_ = ctx0_pieces.pop(0)
            if qt_ == 0:
                cx_cur[(0, h)] = mm_ps.tile(
                    [HD + 1, 512], F32, tag="mm", name=f"cx_0_{h}"
                )
            for j in range(2 * qt_, 2 * qt_ + 2):
                emit_ctx_mm(0, h, j, ats0[h][j], cx_cur[(0, h)])
            if qt_ == 3:
                emit_norm(0, h, cx_cur.pop((0, h)))

        # Chunks 1-3 run as ONE continuous 96-step stream (no chunk seams):
        # ctx pops lag 5 score tiles, and the backfill queue [ctx0 pieces,
        # outproj(0..2) halves] fires evenly across the whole stream.  Each
        # piece kind becomes data-ready just before its queue position.
        # The heads-0/1 half of chunk-3's outproj fires in the last stream
        # steps (norms(3,0/1) complete ~15 steps before the end) targeting
        # out3b, leaving only the heads-2/3 half for the tail.
        pend = deque()
        pieces = [
            (lambda s=s: emit_v_round(s)) for s in range(12, 16)
        ] + [emit_ctx0_piece] * len(ctx0_pieces)
        for pn in range(3):
            pieces += [
                (lambda si=si, nn=nn, pn=pn: emit_outproj_half(pn, si, nn))
                for si in range(4)
                for nn in range(2)
            ]
        late_pieces = [
            (lambda si=si, nn=nn: emit_outproj_half(
                3, si, nn, heads=(0, 1), dst=out3b[ts(si, 128), :], key="b"))
            for si in range(4)
            for nn in range(2)
        ]
        npieces = len(pieces)
        nsteps = 96
        fired = 0
        step = 0
        # units run in interleaved PAIRS so a unit's norm chain (recip ->
        # broadcast -> mul, ~2.5us) overlaps the partner unit's stream
        # instead of stalling the next cx psum allocation.
        for n in range(1, 4):
            for hp in range(2):
                for j in range(8):
                    for h in (2 * hp, 2 * hp + 1):
                        at = emit_scores(n, h, j)
                        pend.append((n, h, j, at))
                        if len(pend) > 5:
                            pop_ctx(pend)
                        # drain the ctx pipeline faster near the very end so
                        # the final norms complete earlier
                        if step >= nsteps - 8 and pend:
                            pop_ctx(pend)
                        step += 1
                        while pieces and fired < step * npieces // (nsteps - 8):
                            pieces.pop(0)()
                            fired += 1
                        if step >= 89 and late_pieces:
                            late_pieces.pop(0)()
        while pend:
            pop_ctx(pend)
        while pieces:
            pieces.pop(0)()
        while late_pieces:
            late_pieces.pop(0)()
        for si in range(4):
            for nn in range(2):
                emit_outproj_half(3, si, nn, act_copies=True, heads=(2, 3))

    nc.compile()
    return nc


def _get_nc():
    if "nc" not in _CACHE:
        _CACHE["nc"] = _build_nc()
    return _CACHE["nc"]


def _make_in_maps(inputs):
    x = np.asarray(inputs["x"], np.float32)
    Wq = np.asarray(inputs["Wq"], np.float32)
    Wk = np.asarray(inputs["Wk"], np.float32)
    Wv = np.asarray(inputs["Wv"], np.float32)
    Wo = np.asarray(inputs["Wo"], np.float32)
    bq = np.asarray(inputs["bq"], np.float32)
    bk = np.asarray(inputs["bk"], np.float32)
    bv = np.asarray(inputs["bv"], np.float32)

    # column permutation for the (h, dh, dp) projection layout:
    # m-tile mt, partition r=(h*32+dh) -> local column 64*h + 2*dh + mt
    r = np.arange(128)
    perm = np.concatenate(
        [64 * (r // 32) + 2 * (r % 32) + mt for mt in range(2)]
    )  # [256] local column index, m-tile-major

    def tile_w(w_slice):
        # [1024, 256] -> permute columns -> [2, 128, 8*128]
        wp = w_slice[:, perm]
        return np.ascontiguousarray(
            wp.reshape(8, 128, 2, 128).transpose(2, 1, 0, 3).reshape(2, 128, 8 * 128)
        ).astype(NP_BF16)

    def tile_b(b_slice):
        return np.ascontiguousarray(b_slice[perm].reshape(2, 128).T)

    def tile_wv(w_slice):
        # [1024, 256] -> [128, 8*256] with k-tile-major free dim
        return np.ascontiguousarray(
            w_slice.reshape(8, 128, C).transpose(1, 0, 2).reshape(128, 8 * C)
        ).astype(NP_BF16)

    in_maps = []
    for core in range(N_CORES):
        b, hg = core // 4, core % 4
        cs = slice(C * hg, C * (hg + 1))
        xT = np.ascontiguousarray(x[b].T).astype(NP_BF16)
        wo_c = np.zeros((HD + 1, HPC * D), np.float32)
        for h in range(HPC):
            r0 = C * hg + HD * h
            wo_c[1 : HD + 1, D * h : D * (h + 1)] = Wo[r0 : r0 + HD]
            wo_c[0, D * h : D * (h + 1)] = bv[r0 : r0 + HD] @ Wo[r0 : r0 + HD]
        in_maps.append(
            {
                "xT": xT,
                "wq": tile_w(Wq[:, cs]),
                "wk": tile_w(Wk[:, cs]),
                "wv": tile_wv(Wv[:, cs]),
                "wo": wo_c.astype(NP_BF16),
                "bq": tile_b(bq[cs]),
                "bk": tile_b(bk[cs]),
            }
        )
    return in_maps


def run(inputs, trace=False):
    """Run the SPMD kernel; returns (full_output, BassKernelResults)."""
    nc = _get_nc()
    in_maps = _make_in_maps(inputs)
    res = bass_utils.run_bass_kernel_spmd(
        nc, in_maps, core_ids=list(range(N_CORES)), trace=trace
    )
    bo = np.asarray(inputs["bo"], np.float32)
    full = np.empty((2, S, D), np.float32)
    for b in range(2):
        acc = res.results[4 * b]["out"].astype(np.float32).copy()
        acc[1536:2048] += res.results[4 * b]["out3b"]
        for hg in range(1, 4):
            acc += res.results[4 * b + hg]["out"]
            acc[1536:2048] += res.results[4 * b + hg]["out3b"]
        full[b] = acc + bo
    return full, res


def kernel(**inputs):
    full, _ = run(inputs, trace=False)
    return full


# revision 46
# speedup vs baseline: 1.0243x; 1.0114x over previous
"""Multi-head attention (B=2, S=2048, D=1024, H=16) on 8 NeuronCores.

Sharding: core = (batch b, head-group hg) with b in {0,1}, hg in {0..3}.
Each core computes 4 heads (256 of the 1024 hidden dims) for one batch
element and produces a partial output [S, D]; the host sums the 4
head-group partials per batch and adds the output bias.

Per-core dataflow:
  Q^T = Wq_c^T @ x^T  (bf16 matmuls, fp32 PSUM) -> quantized fp8e4 on the
        PSUM->SBUF bias-add copy, laid out [128p=(h,dh), 2=dp, S] so the
        scores matmuls can run in fp8 DoubleRow mode (d = 2*dh + dp).
  K^T likewise.
  V   = x @ Wv  (no bias)   [S, 256] bf16
  scores^T[k,q] = per (head, key-tile): ONE fp8e4 DoubleRow matmul
        lhsT = K^T[32, 2, 128], rhs = Q^T[32, 2, 512]  (Ki=32 x 2 planes
        = 64-dim contraction; half cost per output row vs bf16)
  attn^T = exp(scores^T / 8) bf16 (no max subtraction: |s/8| < ~2)
  ctx_aug^T = [ones | V_h]^T @ attn^T  (bf16) -> row 0 = softmax denom
  ctx^T = ctx_aug^T * partition_broadcast(1/denom)
  out_partial = sum_h ctx_h^T slices @ Wo_aug_h  (K=65; Wo row 0 carries
        bv_h @ Wo_h so the ctx 1.0-row adds the V-bias contribution)
Partial outputs are bf16; the host accumulates head groups + bo in fp32.

Host-side input layouts (pre-tiled so every load is one plain 2D DMA):
  xT  [1024, 2048]  x[b].T                                       bf16
  wq/wk [2, 128, 1024]  m-tile-split, k-tile-major columns; the
        columns of each m-tile mt are ordered (h, dh) -> Wq column
        64*h + 2*dh + mt, so the projection PSUM rows land directly in
        the (h, dh) partition layout with mt as the dp plane.         bf16
  wv  [128, 2048]   k-tile-major columns                         bf16
  wo  [65, 4096]    per-head [bv_h @ Wo_h; Wo_h] side by side    bf16
  bq/bk [128, 2]    bias m-tile columns (same column order)      f32
"""

from contextlib import ExitStack

import ml_dtypes
import numpy as np

import concourse.bass as bass
import concourse.mybir as mybir
import concourse.tile as tile
from concourse import bacc
from concourse.bass import ts
from concourse import bass_utils

S = 2048
D = 1024
H = 16
HD = 64
HPC = 4          # heads per core
C = HPC * HD     # 256 hidden dims per core
N_CORES = 8

BF16 = mybir.dt.bfloat16
F32 = mybir.dt.float32
F8 = mybir.dt.float8e4
NP_BF16 = ml_dtypes.bfloat16
NP_F8 = np.dtype(mybir.dt.np(mybir.dt.float8e4))
DR = mybir.MatmulPerfMode.DoubleRow

_CACHE = {}


def _build_nc():
    nc = bacc.Bacc(
        "TRN2", target_bir_lowering=False, debug=False, num_devices=N_CORES
    )

    xT = nc.dram_tensor("xT", [D, S], BF16, kind="ExternalInput").ap()
    wq = nc.dram_tensor("wq", [2, 128, 8 * 128], BF16, kind="ExternalInput").ap()
    wk = nc.dram_tensor("wk", [2, 128, 8 * 128], BF16, kind="ExternalInput").ap()
    wv = nc.dram_tensor("wv", [128, 8 * C], BF16, kind="ExternalInput").ap()
    wo = nc.dram_tensor("wo", [HD + 1, HPC * D], BF16, kind="ExternalInput").ap()
    bq = nc.dram_tensor("bq", [128, 2], F32, kind="ExternalInput").ap()
    bk = nc.dram_tensor("bk", [128, 2], F32, kind="ExternalInput").ap()
    out = nc.dram_tensor("out", [S, D], BF16, kind="ExternalOutput").ap()
    # chunk-3 output rows split by head pair: out3b carries the heads-2/3
    # contribution for rows 1536:2048 (host adds it), so the heads-0/1 half
    # of the final out-projection can run before the last norms complete.
    out3b = nc.dram_tensor("out3b", [512, D], BF16, kind="ExternalOutput").ap()

    with tile.TileContext(nc, pool_alloc_mode="queue") as tc, ExitStack() as ctx:
        ep = ctx.enter_context

        xt_pool = ep(tc.tile_pool(name="xt", bufs=8))
        w_pool = ep(tc.tile_pool(name="w", bufs=5))
        wo_pool = ep(tc.tile_pool(name="wo", bufs=1))
        small_pool = ep(tc.tile_pool(name="small", bufs=4))
        qk_pool = ep(tc.tile_pool(name="qk", bufs=2))
        vaug_pool = ep(tc.tile_pool(name="vaug", bufs=16))
        ctx_pool = ep(tc.tile_pool(name="ctxp", bufs=16))
        attn_pool = ep(tc.tile_pool(name="attn", bufs=38))
        recip_pool = ep(tc.tile_pool(name="recip", bufs=4))
        bcast_pool = ep(tc.tile_pool(name="bcast", bufs=4))
        outsb_pool = ep(tc.tile_pool(name="outsb", bufs=4))
        mm_ps = ep(tc.tile_pool(name="mmps", bufs=2, space="PSUM"))
        sc_ps = ep(tc.tile_pool(name="scps", bufs=2, space="PSUM"))
        cx_ps = ep(tc.tile_pool(name="cxps", bufs=2, space="PSUM"))

        # ---- loads (weights first; xT in k-tiles) ----
        # The first consumers (K00.k0, K01.k0) need only the k=0 weight
        # columns and the first/second q-chunks of xt0, so those land as
        # small head DMAs ahead of the bulk transfers: the first matmul
        # starts ~2us earlier than with whole-tile loads.
        # The warmup K/Q rounds read only columns 0:1024 of each xt k-tile
        # (q-chunks 0-1), so each xt loads as an a-half (critical) and a
        # b-half (deferred until after all a-halves): the warmup-critical
        # DMA stream shrinks from ~13.5us to ~8.6us.
        wk_sb = [None, None]
        wq_sb = [None, None]
        # the first transfers issue from BOTH hwdge queues (SP + ACT) so
        # their fixed per-DMA pipeline heads overlap
        for m in range(2):
            wk_sb[m] = w_pool.tile([128, 8 * 128], BF16, tag="w", name=f"wk_sb{m}")
            nc.sync.dma_start(wk_sb[m][:, 0:128], wk[m][:, 0:128])
        xt = [xt_pool.tile([128, S], BF16, tag="xt", name=f"xt_{k}") for k in range(8)]
        nc.scalar.dma_start(xt[0][:, 0:1024], xT[ts(0, 128), 0:1024])
        for m in range(2):
            nc.sync.dma_start(wk_sb[m][:, 128:1024], wk[m][:, 128:1024])
        nc.sync.dma_start(xt[1][:, 0:1024], xT[ts(1, 128), 0:1024])
        bk_sb = small_pool.tile([128, 2], F32, tag="bqk", name="bk_sb")
        nc.sync.dma_start(bk_sb[:], bk[:])
        bq_sb = small_pool.tile([128, 2], F32, tag="bqk", name="bq_sb")
        nc.sync.dma_start(bq_sb[:], bq[:])
        # wq heads cover k-steps 0-1; Q warmup rounds lag 3 k-steps so the
        # wq head/bulk transfers can trail the early xt a-halves without
        # stalling the PE
        for m in range(2):
            wq_sb[m] = w_pool.tile([128, 8 * 128], BF16, tag="w", name=f"wq_sb{m}")
            nc.scalar.dma_start(wq_sb[m][:, 0:256], wq[m][:, 0:256])
        for k in range(2, 5):
            nc.sync.dma_start(xt[k][:, 0:1024], xT[ts(k, 128), 0:1024])
        for m in range(2):
            nc.sync.dma_start(wq_sb[m][:, 256:1024], wq[m][:, 256:1024])
        for k in range(5, 8):
            nc.sync.dma_start(xt[k][:, 0:1024], xT[ts(k, 128), 0:1024])
        wv_sb = w_pool.tile([128, 8 * C], BF16, tag="w", name="wv_sb")
        nc.sync.dma_start(wv_sb[:], wv[:])
        for k in range(8):
            nc.sync.dma_start(xt[k][:, 1024:2048], xT[ts(k, 128), 1024:2048])
        wo_sb = wo_pool.tile([HD + 1, HPC * D], BF16, tag="wo", name="wo_sb")
        nc.sync.dma_start(wo_sb[:], wo[:])

        # fp8 K^T/Q^T tiles: [128 = (h, dh), 2 = dp, 2048 = s]
        kt = qk_pool.tile([128, 2 * S], F8, tag="qk", name="kt")
        qt = qk_pool.tile([128, 2 * S], F8, tag="qk", name="qt")
        kt3 = kt[:].rearrange("p (two s) -> p two s", two=2)
        qt3 = qt[:].rearrange("p (two s) -> p two s", two=2)

        # ---- projection emitters ----
        def emit_kq_round(dst3, w_t, b_t, m, n, label, pool=None, tag="mm"):
            ps = (pool or mm_ps).tile(
                [128, 512], F32, tag=tag, name=f"ps{label}_{m}_{n}"
            )
            for k in range(8):
                nc.tensor.matmul(
                    ps[:],
                    lhsT=w_t[m][:, ts(k, 128)],
                    rhs=xt[k][:, ts(n, 512)],
                    start=(k == 0),
                    stop=(k == 7),
                )
            nc.vector.tensor_scalar(
                dst3[:, m, ts(n, 512)],
                ps[:],
                b_t[:, m : m + 1],
                None,
                mybir.AluOpType.add,
            )

        vaug = []

        # ---- attention unit: one (q-chunk n, head h) ----
        ctx_tiles = {}

        def emit_scores(n, h, j, pool=None, tag="sc"):
            """fp8 DoubleRow scores for key tiles t=2j,2j+1 + exp -> at."""
            sc = (pool or sc_ps).tile(
                [128, 1024], F32, tag=tag, name=f"sc_{n}_{h}_{j}"
            )
            for tt in range(2):
                t = 2 * j + tt
                nc.tensor.matmul(
                    sc[:, ts(tt, 512)],
                    lhsT=kt3[32 * h : 32 * h + 32, :, ts(t, 128)],
                    rhs=qt3[32 * h : 32 * h + 32, :, ts(n, 512)],
                    start=True,
                    stop=True,
                    perf_mode=DR,
                    tile_position=(32 * h, 0),
                )
            at = attn_pool.tile([128, 1024], BF16, tag="at", name=f"at_{n}_{h}_{j}")
            nc.scalar.activation(
                at[:],
                sc[:],
                mybir.ActivationFunctionType.Exp,
                scale=0.125,
            )
            return at

        def emit_ctx_mm(n, h, j, at, cx):
            for tt in range(2):
                t = 2 * j + tt
                nc.tensor.matmul(
                    cx[:],
                    lhsT=vaug[t][:, 65 * h : 65 * h + 65],
                    rhs=at[:, ts(tt, 512)],
                    start=(t == 0),
                    stop=(t == 15),
                )

        ones65 = small_pool.tile([1, HD + 1], F32, tag="ones", name="ones65")
        nc.vector.memset(ones65[:], 1.0)
        F32R = mybir.dt.float32r

        def emit_norm(n, h, cx, pe_bcast=False):
            rc = recip_pool.tile([1, 512], F32, tag="rc", name=f"rc_{n}_{h}")
            nc.vector.reciprocal(rc[:], cx[0:1, :])
            if pe_bcast:
                # tail units: broadcast via a K=1 fp32r matmul on the (idle)
                # PE instead of GPSIMD — shorter critical chain into the
                # final out-projection.
                bc = mm_ps.tile([HD + 1, 512], F32, tag="mm", name=f"bc_{n}_{h}")
                nc.tensor.matmul(
                    bc[:],
                    lhsT=ones65[:].bitcast(F32R),
                    rhs=rc[:].bitcast(F32R),
                    start=True,
                    stop=True,
                )
            else:
                bc = bcast_pool.tile(
                    [HD + 1, 512], F32, tag="bc", name=f"bc_{n}_{h}"
                )
                nc.gpsimd.partition_broadcast(bc[:], rc[:], channels=HD + 1)
            ct = ctx_pool.tile([HD + 1, 512], BF16, tag="ctx", name=f"ctx_{n}_{h}")
            nc.vector.tensor_mul(ct[:], cx[:], bc[:])
            ctx_tiles[(h, n)] = ct

        ob_open = {}

        def emit_outproj_half(n, si, nn, act_copies=False, heads=range(HPC),
                              dst=None, key=None):
            """One outproj psum group (853ns PE): rows s=4n+si, D-half nn,
            summed over `heads`, written to dram `dst` (default: out).
            The output row-block DMAs in two half-width transfers so the
            tail drain starts as soon as the first half's copy lands."""
            s = 4 * n + si
            if dst is None:
                dst = out[ts(s, 128), :]
            key = (key, s)
            if nn == 0:
                ob_open[key] = outsb_pool.tile(
                    [128, D], BF16, tag="ob", name=f"ob_{key[0]}_{s}"
                )
            ob = ob_open[key]
            pool, tag = (mm_ps, "mm")
            if act_copies:
                pool, tag = (sc_ps, "sc") if nn == 0 else (cx_ps, "cx")
            ps = pool.tile([128, 512], F32, tag=tag, name=f"pso_{key[0]}_{s}_{nn}")
            heads = list(heads)
            for h in heads:
                nc.tensor.matmul(
                    ps[:],
                    lhsT=ctx_tiles[(h, n)][:, ts(si, 128)],
                    rhs=wo_sb[
                        :, 1024 * h + 512 * nn : 1024 * h + 512 * nn + 512
                    ],
                    start=(h == heads[0]),
                    stop=(h == heads[-1]),
                )
            if act_copies and nn == 1:
                nc.scalar.copy(ob[:, ts(nn, 512)], ps[:])
            else:
                nc.vector.tensor_copy(ob[:, ts(nn, 512)], ps[:])
            # one full-width DMA per row block: the DMA queue is per-transfer
            # overhead-dominated, so fewer/bigger beats earlier/smaller
            if nn == 1:
                nc.sync.dma_start(dst[:], ob[:])
                del ob_open[key]

        # ---- emission order ----
        # ACT table-load warm: dummy exp as soon as wk0 lands, so the ~1.3us
        # table load overlaps the xT DMA stream instead of the first scores.
        warm = small_pool.tile([1, 8], BF16, tag="warm", name="actwarm")
        nc.scalar.activation(
            warm[:],
            wk_sb[0][0:1, 0:8],
            mybir.ActivationFunctionType.Exp,
            scale=0.125,
        )

        # Warmup: six K/Q rounds accumulate k-MAJOR so the PE tracks the xT
        # DMA stream (6 matmuls ready per xt tile arrival). Q rounds lag one
        # k-step because wq lands after xt0.
        wu = [
            ("K00", wk_sb[0], kt3, bk_sb, 0, 0, mm_ps, "mm"),
            ("K10", wk_sb[1], kt3, bk_sb, 1, 0, cx_ps, "cx"),
            ("K01", wk_sb[0], kt3, bk_sb, 0, 1, mm_ps, "mm"),
            ("K11", wk_sb[1], kt3, bk_sb, 1, 1, cx_ps, "cx"),
            ("Q00", wq_sb[0], qt3, bq_sb, 0, 0, sc_ps, "sc"),
            ("Q10", wq_sb[1], qt3, bq_sb, 1, 0, sc_ps, "sc"),
        ]
        wu_ps = {
            nm: pool.tile([128, 512], F32, tag=tag, name=f"wu{nm}")
            for (nm, _, _, _, _, _, pool, tag) in wu
        }
        for k in range(11):
            for nm, w_t, _, _, m, n, _, _ in wu:
                kk = k - 3 if nm[0] == "Q" else k
                if not (0 <= kk < 8):
                    continue
                nc.tensor.matmul(
                    wu_ps[nm][:],
                    lhsT=w_t[:, ts(kk, 128)],
                    rhs=xt[kk][:, ts(n, 512)],
                    start=(kk == 0),
                    stop=(kk == 7),
                )
        # bias-add copies split across DVE and ACT so the first scores
        # (which need K00/K10/Q00/Q10) unblock after ~2 copies per engine.
        wu_cp_order = {"K00": 0, "K10": 1, "Q00": 2, "Q10": 3, "K01": 4, "K11": 5}
        for nm, _, dst3, b_t, m, n, _, _ in sorted(
            wu, key=lambda e: wu_cp_order[e[0]]
        ):
            dst = dst3[:, m, ts(n, 512)]
            if nm in ("K10", "Q10", "K01"):
                nc.scalar.activation(
                    dst,
                    wu_ps[nm][:],
                    mybir.ActivationFunctionType.Identity,
                    bias=b_t[:, m : m + 1],
                )
            else:
                nc.vector.tensor_scalar(
                    dst,
                    wu_ps[nm][:],
                    b_t[:, m : m + 1],
                    None,
                    mybir.AluOpType.add,
                )

        # Chunk-0 scores stream with the remaining 10 K/Q rounds and the 16
        # V rounds as PE backfill between ACT-gated score tiles.
        ats0 = {h: [] for h in range(HPC)}
        kq_backfill = [
            ("q", 0, 1), ("q", 1, 1),
            ("k", 0, 2), ("k", 1, 2),
            ("q", 0, 2), ("q", 1, 2),
            ("k", 0, 3), ("k", 1, 3),
            ("q", 0, 3), ("q", 1, 3),
        ]
        v_emitted = 0

        def emit_v_round(s):
            ps = mm_ps.tile([128, C], F32, tag="mm", name=f"psv_{s}")
            for k in range(8):
                nc.tensor.matmul(
                    ps[:],
                    lhsT=xt[k][:, ts(s, 128)],
                    rhs=wv_sb[:, ts(k, C)],
                    start=(k == 0),
                    stop=(k == 7),
                )
            vt = vaug_pool.tile(
                [128, HPC * (HD + 1)], BF16, tag="vaug", name=f"vaug_{s}"
            )
            vt3 = vt[:].rearrange("p (h x) -> p h x", x=HD + 1)
            nc.vector.memset(vt3[:, :, 0:1], 1.0)
            nc.vector.tensor_copy(
                vt3[:, :, 1 : HD + 1],
                ps[:].rearrange("p (h d) -> p h d", d=HD),
            )
            vaug.append(vt)

        bf_i = 0

        def pop_backfill():
            nonlocal bf_i, v_emitted
            if bf_i < len(kq_backfill):
                kind, m, nn = kq_backfill[bf_i]
                pool, tag = (mm_ps, "mm") if bf_i % 2 == 0 else (cx_ps, "cx")
                if kind == "k":
                    emit_kq_round(kt3, wk_sb, bk_sb, m, nn, "k", pool=pool, tag=tag)
                else:
                    emit_kq_round(qt3, wq_sb, bq_sb, m, nn, "q", pool=pool, tag=tag)
                bf_i += 1
            elif v_emitted < 16:
                emit_v_round(v_emitted)
                v_emitted += 1

        # K-round backfill for kt chunk nn pops before the j = 2*nn scores
        # need it.  The last 4 V rounds move into the main stream's piece
        # queue so the chunk-0 phase tail doesn't starve ACT.
        for j in range(8):
            for h in range(HPC):
                ats0[h].append(emit_scores(0, h, j))
                if h % 2 == 1:
                    pop_backfill()
        while bf_i < len(kq_backfill) or v_emitted < 8:
            pop_backfill()

        # Chunks 1-3: software-pipelined scores/ctx (ctx lags 2 score tiles
        # so exp has drained), with chunk-0 ctx and deferred outproj pieces
        # as additional PE backfill spread through the stream.
        from collections import deque

        cx_cur = {}

        def pop_ctx(pend):
            n, h, j, at = pend.popleft()
            if j == 0:
                cx_cur[(n, h)] = cx_ps.tile(
                    [HD + 1, 512], F32, tag="cx", name=f"cx_{n}_{h}"
                )
            emit_ctx_mm(n, h, j, at, cx_cur[(n, h)])
            if j == 7:
                emit_norm(n, h, cx_cur.pop((n, h)))

        # chunk-0 ctx units as backfill pieces for chunk 1 (quarter-unit per
        # piece, 4 matmuls ~850ns); they are all exp-complete by now.
        ctx0_pieces = []
        for h in range(HPC):
            for qt_ in range(4):
                ctx0_pieces.append((h, qt_))

        def emit_ctx0_piece():
            # ctx0 backfills chunk 1, whose pair-interleaved units occupy
            # both cx slots — use the (idle in chunk 1) mm pool instead.
            h, qt_ = ctx0_pieces.pop(0)
            if qt_ == 0:
                cx_cur[(0, h)] = mm_ps.tile(
                    [HD + 1, 512], F32, tag="mm", name=f"cx_0_{h}"
                )
            for j in range(2 * qt_, 2 * qt_ + 2):
                emit_ctx_mm(0, h, j, ats0[h][j], cx_cur[(0, h)])
            if qt_ == 3:
                emit_norm(0, h, cx_cur.pop((0, h)))

        # Chunks 1-3 run as ONE continuous 96-step stream (no chunk seams):
        # ctx pops lag 5 score tiles, and the backfill queue [ctx0 pieces,
        # outproj(0..2) halves] fires evenly across the whole stream.  Each
        # piece kind becomes data-ready just before its queue position.
        # The heads-0/1 half of chunk-3's outproj fires in the last stream
        # steps (norms(3,0/1) complete ~15 steps before the end) targeting
        # out3b, leaving only the heads-2/3 half for the tail.
        pend = deque()
        pieces = [
            (lambda s=s: emit_v_round(s)) for s in range(8, 16)
        ] + [emit_ctx0_piece] * len(ctx0_pieces)
        for pn in range(3):
            pieces += [
                (lambda si=si, nn=nn, pn=pn: emit_outproj_half(pn, si, nn))
                for si in range(4)
                for nn in range(2)
            ]
        late_pieces = [
            (lambda si=si, nn=nn: emit_outproj_half(
                3, si, nn, heads=(0, 1), dst=out3b[ts(si, 128), :], key="b"))
            for si in range(4)
            for nn in range(2)
        ]
        npieces = len(pieces)
        nsteps = 96
        fired = 0
        step = 0
        # units run in interleaved PAIRS so a unit's norm chain (recip ->
        # broadcast -> mul, ~2.5us) overlaps the partner unit's stream
        # instead of stalling the next cx psum allocation.
        for n in range(1, 4):
            for hp in range(2):
                for j in range(8):
                    for h in (2 * hp, 2 * hp + 1):
                        at = emit_scores(n, h, j)
                        pend.append((n, h, j, at))
                        if len(pend) > 5:
                            pop_ctx(pend)
                        # drain the ctx pipeline faster near the very end so
                        # the final norms complete earlier
                        if step >= nsteps - 8 and pend:
                            pop_ctx(pend)
                        step += 1
                        while pieces and fired < step * npieces // (nsteps - 8):
                            pieces.pop(0)()
                            fired += 1
                        if step >= 89 and late_pieces:
                            late_pieces.pop(0)()
        while pend:
            pop_ctx(pend)
        while pieces:
            pieces.pop(0)()
        while late_pieces:
            late_pieces.pop(0)()
        for si in range(4):
            for nn in range(2):
                emit_outproj_half(3, si, nn, act_copies=True, heads=(2, 3))

    nc.compile()
    return nc


def _get_nc():
    if "nc" not in _CACHE:
        _CACHE["nc"] = _build_nc()
    return _CACHE["nc"]


def _make_in_maps(inputs):
    x = np.asarray(inputs["x"], np.float32)
    Wq = np.asarray(inputs["Wq"], np.float32)
    Wk = np.asarray(inputs["Wk"], np.float32)
    Wv = np.asarray(inputs["Wv"], np.float32)
    Wo = np.asarray(inputs["Wo"], np.float32)
    bq = np.asarray(inputs["bq"], np.float32)
    bk = np.asarray(inputs["bk"], np.float32)
    bv = np.asarray(inputs["bv"], np.float32)

    # column permutation for the (h, dh, dp) projection layout:
    # m-tile mt, partition r=(h*32+dh) -> local column 64*h + 2*dh + mt
    r = np.arange(128)
    perm = np.concatenate(
        [64 * (r // 32) + 2 * (r % 32) + mt for mt in range(2)]
    )  # [256] local column index, m-tile-major

    def tile_w(w_slice):
        # [1024, 256] -> permute columns -> [2, 128, 8*128]
        wp = w_slice[:, perm]
        return np.ascontiguousarray(
            wp.reshape(8, 128, 2, 128).transpose(2, 1, 0, 3).reshape(2, 128, 8 * 128)
        ).astype(NP_BF16)

    def tile_b(b_slice):
        return np.ascontiguousarray(b_slice[perm].reshape(2, 128).T)

    def tile_wv(w_slice):
        # [1024, 256] -> [128, 8*256] with k-tile-major free dim
        return np.ascontiguousarray(
            w_slice.reshape(8, 128, C).transpose(1, 0, 2).reshape(128, 8 * C)
        ).astype(NP_BF16)

    in_maps = []
    for core in range(N_CORES):
        b, hg = core // 4, core % 4
        cs = slice(C * hg, C * (hg + 1))
        xT = np.ascontiguousarray(x[b].T).astype(NP_BF16)
        wo_c = np.zeros((HD + 1, HPC * D), np.float32)
        for h in range(HPC):
            r0 = C * hg + HD * h
            wo_c[1 : HD + 1, D * h : D * (h + 1)] = Wo[r0 : r0 + HD]
            wo_c[0, D * h : D * (h + 1)] = bv[r0 : r0 + HD] @ Wo[r0 : r0 + HD]
        in_maps.append(
            {
                "xT": xT,
                "wq": tile_w(Wq[:, cs]),
                "wk": tile_w(Wk[:, cs]),
                "wv": tile_wv(Wv[:, cs]),
                "wo": wo_c.astype(NP_BF16),
                "bq": tile_b(bq[cs]),
                "bk": tile_b(bk[cs]),
            }
        )
    return in_maps


def run(inputs, trace=False):
    """Run the SPMD kernel; returns (full_output, BassKernelResults)."""
    nc = _get_nc()
    in_maps = _make_in_maps(inputs)
    res = bass_utils.run_bass_kernel_spmd(
        nc, in_maps, core_ids=list(range(N_CORES)), trace=trace
    )
    bo = np.asarray(inputs["bo"], np.float32)
    full = np.empty((2, S, D), np.float32)
    for b in range(2):
        acc = res.results[4 * b]["out"].astype(np.float32).copy()
        acc[1536:2048] += res.results[4 * b]["out3b"]
        for hg in range(1, 4):
            acc += res.results[4 * b + hg]["out"]
            acc[1536:2048] += res.results[4 * b + hg]["out3b"]
        full[b] = acc + bo
    return full, res


def kernel(**inputs):
    full, _ = run(inputs, trace=False)
    return full
